# revision 12
# baseline (speedup 1.0000x reference)
"""Causal self-attention (B=4, T=2048, E=2048, H=16) on 8 trn2 NeuronCores.

Tensor-parallel over heads: 2 heads per core. Per-core Bass/Tile kernel:
  qkvT = w_qkvT.T @ xT (bf16 matmuls, f32 PSUM), fused rotate-half RoPE
  (DVE), attention in transposed layout (scoresT = k.T@q so softmax'd
  probs feed the PV matmul directly, no transposes of P), causal block
  skipping with narrowed diagonal tiles, softmax without max-subtraction
  (scores are O(5), exp cannot overflow), both heads software-pipelined
  (PV lags scores by one step so exp latency is hidden).

  The softmax denominator comes from a ones-column matmul accumulated
  next to PV (offloading it to DVE/GPSIMD adds was tried and is SLOWER:
  the serial per-qt accumulation chains pace the whole pipeline and the
  extra engine activity deepens the power throttle).

  Everything the PE streams is bf16 (x, w_qkv, q, k, v, probs, w_o,
  attention output): fp32r runs at the same cycles/row but doubles SBUF
  read bandwidth and HBM traffic, which feeds the power throttle. w_o
  is resident in SBUF (64 KB/partition), its load spread across batch
  0's compute so no DMA burst trips the 50%-throttle. x is bf16 in HBM.

  Token resharding via EIGHT bf16 AllToAlls: each batch's tokens are
  split in two 1024-token halves (128-token blocks interleaved across
  ranks: block g of a batch -> rank g%8, half g//8), the first half's
  A2A fires mid-attention (after qt=1), the second right at attention
  end. o_proj for half 0 runs straight after attention (hiding half 1's
  A2A), o_proj for half 1 is interleaved into the next batch's QKV
  tiles, so the end-of-kernel tail is only half-a-batch of o_proj.

Host-side prep in kernel(): transpose x and cast bf16, permute q/k
weight rows so RoPE becomes rotate-half (scores invariant under a shared
d-permutation), fold the 1/sqrt(d) scale into w_q, precompute cos/sin
tables, shard w_qkv by head (bf16), cast w_o to bf16. Device emits bf16;
host upcasts to f32.
"""

import sys

sys.path.insert(0, "/opt/trn_rl_repo")

import ml_dtypes
import numpy as np

B, T, E, H = 4, 2048, 2048, 16
HD = E // H            # 128
NC_ = 8                # cores
HPC = H // NC_         # heads per core
CL = 3 * HPC * HD      # local qkv channels = 768
VOFF = 2 * HPC * 128   # column offset of v channels in wqkvT = 512
BLK = T // NC_         # token block per rank per batch = 256
HB = 128               # tokens per A2A half-block
TT = 512               # token tile
EB = E // 128          # 16 contraction blocks
NBT = T // TT          # 4 token tiles per batch
KB = T // 128          # 16 key blocks per batch

_BUILT = None


def _build(b_run=B):
    import concourse.mybir as mybir
    import concourse.tile as tile
    from concourse import bacc

    f32 = mybir.dt.float32
    f32r = mybir.dt.float32r
    bf16 = mybir.dt.bfloat16
    ACT = mybir.ActivationFunctionType
    MUL = mybir.AluOpType.mult

    BT = b_run * T

    nc = bacc.Bacc("TRN2", target_bir_lowering=False, debug=False,
                   num_devices=NC_)

    xT = nc.dram_tensor("xT", [E, BT], bf16, kind="ExternalInput")
    wqkvT = nc.dram_tensor("wqkvT", [E, CL], bf16, kind="ExternalInput")
    woT = nc.dram_tensor("woT", [E, E], bf16, kind="ExternalInput")
    cosT = nc.dram_tensor("cosT", [64, T], f32, kind="ExternalInput")
    sinT = nc.dram_tensor("sinT", [64, T], f32, kind="ExternalInput")
    trimask = nc.dram_tensor("trimask", [128, 128], bf16,
                             kind="ExternalInput")
    outT = nc.dram_tensor("outT", [E, b_run * BLK], bf16,
                          kind="ExternalOutput")

    xT_r = xT.rearrange("(eh p) t -> p eh t", p=128)
    woT_r = woT.rearrange("(cb p) e -> p cb e", p=128)
    wqkvT_r = wqkvT.rearrange("(eb p) c -> p eb c", p=128)

    with tile.TileContext(nc) as tc:
        with tc.tile_pool(name="consts", bufs=1) as consts, \
             tc.tile_pool(name="dram", bufs=1, space="DRAM") as dram, \
             tc.tile_pool(name="wq", bufs=1) as wq_pool, \
             tc.tile_pool(name="xt", bufs=8) as xt_pool, \
             tc.tile_pool(name="qk", bufs=1) as qk_pool, \
             tc.tile_pool(name="pt", bufs=4) as pt_pool, \
             tc.tile_pool(name="eps", bufs=1) as eps_pool, \
             tc.tile_pool(name="wo", bufs=1) as wo_pool, \
             tc.tile_pool(name="oo", bufs=2) as oo_pool, \
             tc.tile_pool(name="bps", bufs=2, space="PSUM") as bps, \
             tc.tile_pool(name="ops_o", bufs=1, space="PSUM") as ops_o, \
             tc.tile_pool(name="pps", bufs=2, space="PSUM") as pps, \
             tc.tile_pool(name="zps", bufs=1, space="PSUM") as zps:
            cos_sb = consts.tile([64, T], f32)
            sin_sb = consts.tile([64, T], f32)
            tri_sb = consts.tile([128, 128], bf16)
            ones_mat = consts.tile([128, 128], bf16)
            nc.sync.dma_start(out=tri_sb[:], in_=trimask[:])
            nc.vector.memset(ones_mat[:], 1.0)

            agl = [[dram.tile([E, HB], bf16, name=f"agl{b}_{h}")
                    for h in range(2)] for b in range(b_run)]
            agf = [[dram.tile([E, HB], bf16, name=f"agf{b}_{h}")
                    for h in range(2)] for b in range(b_run)]

            xcache = {}

            def load_xtile(b, tt):
                if (b, tt) in xcache:
                    return xcache.pop((b, tt))
                t0 = b * T + tt * TT
                xc = []
                for h in range(4):
                    xch = xt_pool.tile([128, EB // 4, TT], bf16, tag="xt",
                                       name="xch")
                    nc.sync.dma_start(
                        out=xch[:],
                        in_=xT_r[:, h * 4:(h + 1) * 4, t0:t0 + TT])
                    xc.append(xch)
                return xc

            # startup: interleave w_qkv blocks, the first x tile and the
            # first cos/sin slices so the first QKV chain starts ASAP.
            w_sb = wq_pool.tile([128, EB, CL], bf16)
            x00 = []

            def _w(e, c0, c1):
                nc.gpsimd.dma_start(out=w_sb[:, e, c0:c1],
                                    in_=wqkvT_r[:, e, c0:c1])

            def _x(h):
                xch = xt_pool.tile([128, EB // 4, TT], bf16, tag="xt",
                                   name="xch")
                nc.sync.dma_start(out=xch[:], in_=xT_r[:, h * 4:(h + 1) * 4,
                                                       0:TT])
                x00.append(xch)

            def _cs(tt):
                sl = slice(tt * TT, (tt + 1) * TT)
                nc.sync.dma_start(out=cos_sb[:, sl], in_=cosT[:, sl])
                nc.sync.dma_start(out=sin_sb[:, sl], in_=sinT[:, sl])

            # column-group-major weight load: the first q/k chains only
            # touch columns 0:256, so the PE starts ~2us in instead of
            # waiting out the full 3 MB weight load
            _x(0)
            _cs(0)
            for e in range(EB):
                _w(e, 0, 256)
            _x(1)
            for e in range(EB):
                _w(e, 256, 512)
            _x(2)
            for e in range(EB):
                _w(e, 512, CL)
            _x(3)
            for tt in range(1, NBT):
                _cs(tt)
            xcache[(0, 0)] = x00
            xcache[(0, 1)] = load_xtile(0, 1)

            # w_o resident in SBUF for the whole kernel; its 8 MB load is
            # trickled out on the gpsimd queue across batch 0's compute so
            # the DMA burst never trips the activity power-throttle.
            wo_sb = wo_pool.tile([128, EB, E], bf16)
            wo_cb_iter = iter(range(EB))

            def wo_feed(n):
                for _ in range(n):
                    cb = next(wo_cb_iter, None)
                    if cb is None:
                        return
                    nc.sync.dma_start(out=wo_sb[:, cb, :],
                                      in_=woT_r[:, cb, :])

            wo_feed(4)

            def wv(e, cs):
                return w_sb[:, e, cs]

            atiles = {}

            def prefetch_atile(b, h):
                a_tile = oo_pool.tile([128, EB, HB], bf16, tag="at",
                                      name="a_tile", bufs=3)
                nc.gpsimd.dma_start(
                    out=a_tile[:],
                    in_=agf[b][h].rearrange("(cb p) t -> p cb t", p=128))
                atiles[(b, h)] = a_tile

            def emit_oproj(b, h, ebs):
                # two ebs share one PSUM tile / ACT copy so the per-chunk
                # PSUM-drain latency amortizes over 2x the matmul work
                a_tile = atiles[(b, h)]
                ebs = list(ebs)
                for i in range(0, len(ebs), 2):
                    pair = ebs[i:i + 2]
                    pso = pps.tile([128, len(pair) * HB], f32, tag="pso",
                                   name="pso")
                    for pi, eb in enumerate(pair):
                        for cb in range(EB):
                            nc.tensor.matmul(
                                pso[:, pi * HB:(pi + 1) * HB],
                                wo_sb[:, cb, eb * 128:(eb + 1) * 128],
                                a_tile[:, cb, :],
                                start=(cb == 0), stop=(cb == EB - 1))
                    ot = oo_pool.tile([128, len(pair) * HB], bf16, tag="ot",
                                      name="ot", bufs=2)
                    nc.scalar.activation(ot[:], pso[:], ACT.Copy)
                    for pi, eb in enumerate(pair):
                        nc.sync.dma_start(
                            out=outT[eb * 128:(eb + 1) * 128,
                                     b * BLK + h * HB:b * BLK + (h + 1) * HB],
                            in_=ot[:, pi * HB:(pi + 1) * HB])

            for b in range(b_run):
                q_sb = [qk_pool.tile([HD, T], bf16, tag=f"q{j}",
                                     name=f"q_sb{j}") for j in range(HPC)]
                k_sb = [qk_pool.tile([HD, T], bf16, tag=f"k{j}",
                                     name=f"k_sb{j}") for j in range(HPC)]
                v_hold = qk_pool.tile([128, KB, HPC * HD], bf16, tag="vh",
                                      name="v_hold")

                # ---- QKV projection for this batch; previous batch's
                # half-1 o_proj interleaved at tile boundaries ----
                for tt in range(NBT):
                    xc = load_xtile(b, tt)

                    def xv(e, ts=slice(None)):
                        return xc[e // 4][:, e % 4, ts]

                    # q/k channels: c-blocks [q0,k0,q1,k1]
                    for c in range(2 * HPC):
                        j, is_k = c // 2, c % 2
                        ps = bps.tile([128, TT], f32, tag="big", name="ps_qk")
                        for e in range(EB):
                            nc.tensor.matmul(
                                ps[:], wv(e, slice(c * 128, (c + 1) * 128)),
                                xv(e), start=(e == 0), stop=(e == EB - 1))
                        # rotate-half rope out of PSUM
                        cs = cos_sb[:, tt * TT:(tt + 1) * TT]
                        sn = sin_sb[:, tt * TT:(tt + 1) * TT]
                        t1 = eps_pool.tile([128, TT], f32, tag="t1",
                                           name="t1", bufs=2)
                        t2 = eps_pool.tile([128, TT], f32, tag="t2",
                                           name="t2", bufs=2)
                        nc.vector.tensor_mul(t1[0:64, :], ps[0:64, :], cs)
                        nc.vector.tensor_mul(t1[64:128, :], ps[64:128, :], cs)
                        nc.vector.scalar_tensor_tensor(
                            t2[0:64, :], ps[64:128, :], -1.0, sn, MUL, MUL)
                        nc.vector.tensor_mul(t2[64:128, :], ps[0:64, :], sn)
                        dst = (k_sb if is_k else q_sb)[j]
                        nc.vector.tensor_add(
                            dst[:, tt * TT:(tt + 1) * TT], t1[:], t2[:])
                    # v channels, natural (t, d) layout, straight to SBUF
                    for tb in range(TT // 128):
                        psv = bps.tile([128, HPC * HD], f32, tag="big",
                                       name="psv")
                        for e in range(EB):
                            nc.tensor.matmul(
                                psv[:], xv(e, slice(tb * 128, (tb + 1) * 128)),
                                wv(e, slice(VOFF, CL)),
                                start=(e == 0), stop=(e == EB - 1))
                        kb = tt * (TT // 128) + tb
                        nc.scalar.activation(v_hold[:, kb, :], psv[:],
                                             ACT.Copy)
                    if b == 0:
                        wo_feed(3)
                    elif b < b_run - 1 or tt < 2:
                        emit_oproj(b - 1, 1, range(4 * tt, 4 * tt + 4))

                # prefetch next batch's first x tile during attention
                if b + 1 < b_run:
                    xcache[(b + 1, 0)] = load_xtile(b + 1, 0)

                # ---- attention: heads interleaved, PV lags scores ----
                for qt in range(NBT):
                    nkt = (qt + 1) * (TT // 128)
                    ps_o = [ops_o.tile([128, TT], f32, tag=f"o{j}",
                                       name=f"ps_o{j}") for j in range(HPC)]
                    ps_z = [zps.tile([128, TT], f32, tag=f"z{j}",
                                     name=f"ps_z{j}") for j in range(HPC)]
                    pts = {}
                    for kt in range(nkt + 1):
                        if kt < nkt:
                            m = kt - qt * (TT // 128)
                            lo = max(m, 0) * 128
                            for j in range(HPC):
                                ps_s = bps.tile([128, TT], f32, tag="big",
                                                name="ps_s")
                                nc.tensor.matmul(
                                    ps_s[:, lo:],
                                    k_sb[j][:, kt * 128:(kt + 1) * 128],
                                    q_sb[j][:, qt * TT + lo:(qt + 1) * TT],
                                    start=True, stop=True)
                                pt = pt_pool.tile([128, TT], bf16, tag="pt",
                                                  name="pt")
                                nc.scalar.activation(
                                    pt[:, lo:], ps_s[:, lo:], ACT.Exp)
                                if m >= 0:
                                    nc.vector.tensor_mul(
                                        pt[:, lo:lo + 128],
                                        pt[:, lo:lo + 128], tri_sb[:])
                                pts[(j, kt)] = pt
                        if kt > 0:
                            pk = kt - 1
                            lo = max(pk - qt * (TT // 128), 0) * 128
                            for j in range(HPC):
                                pt = pts.pop((j, pk))
                                nc.tensor.matmul(
                                    ps_o[j][:, lo:],
                                    v_hold[:, pk, j * HD:(j + 1) * HD],
                                    pt[:, lo:],
                                    start=(pk == 0), stop=(pk == nkt - 1))
                                nc.tensor.matmul(
                                    ps_z[j][:, lo:],
                                    ones_mat[:],
                                    pt[:, lo:],
                                    start=(pk == 0), stop=(pk == nkt - 1))
                    for j in range(HPC):
                        # Z arrives broadcast across partitions (ones-matrix
                        # stationary), so normalization is recip + mul on DVE
                        zri = eps_pool.tile([128, TT], f32, tag="zri",
                                            name="zri", bufs=2)
                        nc.vector.reciprocal_approx_fast(zri[:], ps_z[j][:])
                        ao = eps_pool.tile([128, TT], bf16, tag="ao",
                                           name="ao", bufs=2)
                        nc.vector.tensor_mul(ao[:], ps_o[j][:], zri[:])
                        # scatter 128-token blocks: global block g of this
                        # batch -> rank g%8, half g//8
                        for s in range(TT // HB):
                            g = qt * (TT // HB) + s
                            h, rr = g // 8, g % 8
                            nc.sync.dma_start(
                                out=agl[b][h][rr * (E // NC_) + j * HD:
                                              rr * (E // NC_) + (j + 1) * HD,
                                              :],
                                in_=ao[:, s * HB:(s + 1) * HB])
                    if qt == 1:
                        nc.gpsimd.collective_compute(
                            "AllToAll", mybir.AluOpType.bypass,
                            replica_groups=[list(range(NC_))],
                            ins=[agl[b][0][:]], outs=[agf[b][0][:]])
                        prefetch_atile(b, 0)

                # fire half 1's A2A, then o_proj half 0 runs under it
                nc.gpsimd.collective_compute(
                    "AllToAll", mybir.AluOpType.bypass,
                    replica_groups=[list(range(NC_))],
                    ins=[agl[b][1][:]], outs=[agf[b][1][:]])
                prefetch_atile(b, 1)
                emit_oproj(b, 0, range(EB))
                if b == b_run - 1 and b > 0:
                    # deferred half of the previous batch's o_proj plugs the
                    # gap while the final half-A2A + a_tile load completes
                    emit_oproj(b - 1, 1, range(8, EB))

            emit_oproj(b_run - 1, 1, range(EB))
    nc.compile()
    return nc


def _prep_inputs(x, freqs, w_qkv, w_o, b_run=B):
    bf16 = ml_dtypes.bfloat16
    xf = np.ascontiguousarray(x, dtype=np.float32).reshape(b_run * T, E)
    xT = np.ascontiguousarray(xf.T.astype(bf16))

    wq = w_qkv[0:E].reshape(H, HD, E)
    wk = w_qkv[E:2 * E].reshape(H, HD, E)
    wvv = w_qkv[2 * E:3 * E].reshape(H, HD, E)
    perm = np.concatenate([np.arange(0, HD, 2), np.arange(1, HD, 2)])
    scale = np.float32(1.0 / np.sqrt(HD))
    wq_p = wq[:, perm, :] * scale
    wk_p = wk[:, perm, :]

    cos = np.cos(freqs.astype(np.float32))
    sin = np.sin(freqs.astype(np.float32))
    cosT = np.ascontiguousarray(cos.T)
    sinT = np.ascontiguousarray(sin.T)
    tri = (np.arange(128)[:, None] <= np.arange(128)[None, :]).astype(bf16)
    w_oT = np.ascontiguousarray(w_o.T.astype(bf16))

    in_maps = []
    for r in range(NC_):
        blocks = []
        for j in range(HPC):
            h = r * HPC + j
            blocks += [wq_p[h].T, wk_p[h].T]
        blocks += [wvv[r * HPC + j].T for j in range(HPC)]
        wqkvT_loc = np.ascontiguousarray(
            np.concatenate(blocks, axis=1).astype(bf16))
        in_maps.append({
            "xT": xT,
            "wqkvT": wqkvT_loc,
            "woT": w_oT,
            "cosT": cosT,
            "sinT": sinT,
            "trimask": tri,
        })
    return in_maps


def kernel(x, freqs, w_qkv, w_o, _trace=False, _b_run=B):
    global _BUILT
    from concourse.bass_utils import run_bass_kernel_spmd

    if _BUILT is None or _BUILT[1] != _b_run:
        _BUILT = (_build(_b_run), _b_run)
    nc = _BUILT[0]

    in_maps = _prep_inputs(np.asarray(x), np.asarray(freqs),
                           np.asarray(w_qkv), np.asarray(w_o), _b_run)
    res = run_bass_kernel_spmd(nc, in_maps, core_ids=list(range(NC_)),
                               trace=_trace)
    # core r owns token block g of each batch iff g%8 == r, stored in its
    # outT at columns [b*BLK + (g//8)*HB, ...)
    out = np.empty((E, _b_run * T), np.float32)
    for r in range(NC_):
        o = res.results[r]["outT"]
        for b in range(_b_run):
            for h in range(2):
                g = 8 * h + r
                out[:, b * T + g * HB:b * T + (g + 1) * HB] = \
                    o[:, b * BLK + h * HB:b * BLK + (h + 1) * HB] \
                    .astype(np.float32)
    out = np.ascontiguousarray(out.T).reshape(_b_run, T, E)
    if _trace:
        kernel.last_results = res
    return out.astype(np.float32, copy=False)


# revision 13
# speedup vs baseline: 1.0094x; 1.0094x over previous
"""Causal self-attention (B=4, T=2048, E=2048, H=16) on 8 trn2 NeuronCores.

Tensor-parallel over heads: 2 heads per core. Per-core Bass/Tile kernel:
  qkvT = w_qkvT.T @ xT (bf16 matmuls, f32 PSUM), fused rotate-half RoPE
  (DVE), attention in transposed layout (scoresT = k.T@q so softmax'd
  probs feed the PV matmul directly, no transposes of P), causal block
  skipping with narrowed diagonal tiles, softmax without max-subtraction
  (scores are O(5), exp cannot overflow), both heads software-pipelined
  (PV lags scores by one step so exp latency is hidden).

  The softmax denominator comes from a ones-column matmul accumulated
  next to PV (offloading it to DVE/GPSIMD adds was tried and is SLOWER:
  the serial per-qt accumulation chains pace the whole pipeline and the
  extra engine activity deepens the power throttle).

  Everything the PE streams is bf16 (x, w_qkv, q, k, v, probs, w_o,
  attention output): fp32r runs at the same cycles/row but doubles SBUF
  read bandwidth and HBM traffic, which feeds the power throttle. w_o
  is resident in SBUF (64 KB/partition), its load spread across batch
  0's compute so no DMA burst trips the 50%-throttle. x is bf16 in HBM.

  Token resharding via EIGHT bf16 AllToAlls: each batch's tokens are
  split in two 1024-token halves (128-token blocks interleaved across
  ranks: block g of a batch -> rank g%8, half g//8), the first half's
  A2A fires mid-attention (after qt=1), the second right at attention
  end. o_proj for half 0 runs straight after attention (hiding half 1's
  A2A), o_proj for half 1 is interleaved into the next batch's QKV
  tiles, so the end-of-kernel tail is only half-a-batch of o_proj.

Host-side prep in kernel(): transpose x and cast bf16, permute q/k
weight rows so RoPE becomes rotate-half (scores invariant under a shared
d-permutation), fold the 1/sqrt(d) scale into w_q, precompute cos/sin
tables, shard w_qkv by head (bf16), cast w_o to bf16. Device emits bf16;
host upcasts to f32.
"""

import sys

sys.path.insert(0, "/opt/trn_rl_repo")

import ml_dtypes
import numpy as np

B, T, E, H = 4, 2048, 2048, 16
HD = E // H            # 128
NC_ = 8                # cores
HPC = H // NC_         # heads per core
CL = 3 * HPC * HD      # local qkv channels = 768
VOFF = 2 * HPC * 128   # column offset of v channels in wqkvT = 512
BLK = T // NC_         # token block per rank per batch = 256
HB = 128               # tokens per A2A half-block
TT = 512               # token tile
EB = E // 128          # 16 contraction blocks
NBT = T // TT          # 4 token tiles per batch
KB = T // 128          # 16 key blocks per batch

_BUILT = None


def _build(b_run=B):
    import concourse.mybir as mybir
    import concourse.tile as tile
    from concourse import bacc

    f32 = mybir.dt.float32
    f32r = mybir.dt.float32r
    bf16 = mybir.dt.bfloat16
    ACT = mybir.ActivationFunctionType
    MUL = mybir.AluOpType.mult

    BT = b_run * T

    nc = bacc.Bacc("TRN2", target_bir_lowering=False, debug=False,
                   num_devices=NC_)

    xT = nc.dram_tensor("xT", [E, BT], bf16, kind="ExternalInput")
    wqkvT = nc.dram_tensor("wqkvT", [E, CL], bf16, kind="ExternalInput")
    woT = nc.dram_tensor("woT", [E, E], bf16, kind="ExternalInput")
    cosT = nc.dram_tensor("cosT", [64, T], f32, kind="ExternalInput")
    sinT = nc.dram_tensor("sinT", [64, T], f32, kind="ExternalInput")
    trimask = nc.dram_tensor("trimask", [128, 128], bf16,
                             kind="ExternalInput")
    outT = nc.dram_tensor("outT", [E, b_run * BLK], bf16,
                          kind="ExternalOutput")

    xT_r = xT.rearrange("(eh p) t -> p eh t", p=128)
    woT_r = woT.rearrange("(cb p) e -> p cb e", p=128)
    wqkvT_r = wqkvT.rearrange("(eb p) c -> p eb c", p=128)

    with tile.TileContext(nc) as tc:
        with tc.tile_pool(name="consts", bufs=1) as consts, \
             tc.tile_pool(name="dram", bufs=1, space="DRAM") as dram, \
             tc.tile_pool(name="wq", bufs=1) as wq_pool, \
             tc.tile_pool(name="xt", bufs=8) as xt_pool, \
             tc.tile_pool(name="qk", bufs=1) as qk_pool, \
             tc.tile_pool(name="pt", bufs=4) as pt_pool, \
             tc.tile_pool(name="eps", bufs=1) as eps_pool, \
             tc.tile_pool(name="wo", bufs=1) as wo_pool, \
             tc.tile_pool(name="oo", bufs=2) as oo_pool, \
             tc.tile_pool(name="bps", bufs=2, space="PSUM") as bps, \
             tc.tile_pool(name="ops_o", bufs=1, space="PSUM") as ops_o, \
             tc.tile_pool(name="pps", bufs=2, space="PSUM") as pps, \
             tc.tile_pool(name="zps", bufs=1, space="PSUM") as zps:
            cos_sb = consts.tile([64, T], f32)
            sin_sb = consts.tile([64, T], f32)
            tri_sb = consts.tile([128, 128], bf16)
            ones_mat = consts.tile([128, 128], bf16)
            nc.sync.dma_start(out=tri_sb[:], in_=trimask[:])
            nc.vector.memset(ones_mat[:], 1.0)

            agl = [[dram.tile([E, HB], bf16, name=f"agl{b}_{h}")
                    for h in range(2)] for b in range(b_run)]
            agf = [[dram.tile([E, HB], bf16, name=f"agf{b}_{h}")
                    for h in range(2)] for b in range(b_run)]

            xcache = {}

            def load_xtile(b, tt):
                if (b, tt) in xcache:
                    return xcache.pop((b, tt))
                t0 = b * T + tt * TT
                xc = []
                for h in range(4):
                    xch = xt_pool.tile([128, EB // 4, TT], bf16, tag="xt",
                                       name="xch")
                    nc.sync.dma_start(
                        out=xch[:],
                        in_=xT_r[:, h * 4:(h + 1) * 4, t0:t0 + TT])
                    xc.append(xch)
                return xc

            # startup: interleave w_qkv blocks, the first x tile and the
            # first cos/sin slices so the first QKV chain starts ASAP.
            w_sb = wq_pool.tile([128, EB, CL], bf16)
            x00 = []

            def _w(e):
                nc.gpsimd.dma_start(out=w_sb[:, e, :], in_=wqkvT_r[:, e, :])

            def _x(h):
                xch = xt_pool.tile([128, EB // 4, TT], bf16, tag="xt",
                                   name="xch")
                nc.sync.dma_start(out=xch[:], in_=xT_r[:, h * 4:(h + 1) * 4,
                                                       0:TT])
                x00.append(xch)

            def _cs(tt):
                sl = slice(tt * TT, (tt + 1) * TT)
                nc.sync.dma_start(out=cos_sb[:, sl], in_=cosT[:, sl])
                nc.sync.dma_start(out=sin_sb[:, sl], in_=sinT[:, sl])

            _w(0)
            _x(0)
            _cs(0)
            for e in range(1, 4):
                _w(e)
            _x(1)
            for e in range(4, 8):
                _w(e)
            _x(2)
            for e in range(8, 12):
                _w(e)
            _x(3)
            for e in range(12, EB):
                _w(e)
            for tt in range(1, NBT):
                _cs(tt)
            xcache[(0, 0)] = x00
            xcache[(0, 1)] = load_xtile(0, 1)

            # w_o resident in SBUF for the whole kernel; its 8 MB load is
            # trickled out on the gpsimd queue across batch 0's compute so
            # the DMA burst never trips the activity power-throttle.
            wo_sb = wo_pool.tile([128, EB, E], bf16)
            wo_cb_iter = iter(range(EB))

            def wo_feed(n):
                for _ in range(n):
                    cb = next(wo_cb_iter, None)
                    if cb is None:
                        return
                    nc.sync.dma_start(out=wo_sb[:, cb, :],
                                      in_=woT_r[:, cb, :])

            wo_feed(4)

            def wv(e, cs):
                return w_sb[:, e, cs]

            atiles = {}

            def prefetch_atile(b, h):
                a_tile = oo_pool.tile([128, EB, HB], bf16, tag="at",
                                      name="a_tile", bufs=3)
                nc.gpsimd.dma_start(
                    out=a_tile[:],
                    in_=agf[b][h].rearrange("(cb p) t -> p cb t", p=128))
                atiles[(b, h)] = a_tile

            def emit_oproj(b, h, ebs):
                # two ebs share one PSUM tile / ACT copy so the per-chunk
                # PSUM-drain latency amortizes over 2x the matmul work
                a_tile = atiles[(b, h)]
                ebs = list(ebs)
                for i in range(0, len(ebs), 2):
                    pair = ebs[i:i + 2]
                    pso = pps.tile([128, len(pair) * HB], f32, tag="pso",
                                   name="pso")
                    for pi, eb in enumerate(pair):
                        for cb in range(EB):
                            nc.tensor.matmul(
                                pso[:, pi * HB:(pi + 1) * HB],
                                wo_sb[:, cb, eb * 128:(eb + 1) * 128],
                                a_tile[:, cb, :],
                                start=(cb == 0), stop=(cb == EB - 1))
                    ot = oo_pool.tile([128, len(pair) * HB], bf16, tag="ot",
                                      name="ot", bufs=2)
                    nc.scalar.activation(ot[:], pso[:], ACT.Copy)
                    for pi, eb in enumerate(pair):
                        nc.sync.dma_start(
                            out=outT[eb * 128:(eb + 1) * 128,
                                     b * BLK + h * HB:b * BLK + (h + 1) * HB],
                            in_=ot[:, pi * HB:(pi + 1) * HB])

            for b in range(b_run):
                q_sb = [qk_pool.tile([HD, T], bf16, tag=f"q{j}",
                                     name=f"q_sb{j}") for j in range(HPC)]
                k_sb = [qk_pool.tile([HD, T], bf16, tag=f"k{j}",
                                     name=f"k_sb{j}") for j in range(HPC)]
                v_hold = qk_pool.tile([128, KB, HPC * HD], bf16, tag="vh",
                                      name="v_hold")

                # ---- QKV projection for this batch; previous batch's
                # half-1 o_proj interleaved at tile boundaries ----
                for tt in range(NBT):
                    xc = load_xtile(b, tt)

                    def xv(e, ts=slice(None)):
                        return xc[e // 4][:, e % 4, ts]

                    # q/k channels: c-blocks [q0,k0,q1,k1]
                    for c in range(2 * HPC):
                        j, is_k = c // 2, c % 2
                        ps = bps.tile([128, TT], f32, tag="big", name="ps_qk")
                        for e in range(EB):
                            nc.tensor.matmul(
                                ps[:], wv(e, slice(c * 128, (c + 1) * 128)),
                                xv(e), start=(e == 0), stop=(e == EB - 1))
                        # rotate-half rope out of PSUM
                        cs = cos_sb[:, tt * TT:(tt + 1) * TT]
                        sn = sin_sb[:, tt * TT:(tt + 1) * TT]
                        t1 = eps_pool.tile([128, TT], f32, tag="t1",
                                           name="t1", bufs=2)
                        t2 = eps_pool.tile([128, TT], f32, tag="t2",
                                           name="t2", bufs=2)
                        nc.vector.tensor_mul(t1[0:64, :], ps[0:64, :], cs)
                        nc.vector.tensor_mul(t1[64:128, :], ps[64:128, :], cs)
                        nc.vector.scalar_tensor_tensor(
                            t2[0:64, :], ps[64:128, :], -1.0, sn, MUL, MUL)
                        nc.vector.tensor_mul(t2[64:128, :], ps[0:64, :], sn)
                        dst = (k_sb if is_k else q_sb)[j]
                        nc.vector.tensor_add(
                            dst[:, tt * TT:(tt + 1) * TT], t1[:], t2[:])
                    # v channels, natural (t, d) layout, straight to SBUF
                    for tb in range(TT // 128):
                        psv = bps.tile([128, HPC * HD], f32, tag="big",
                                       name="psv")
                        for e in range(EB):
                            nc.tensor.matmul(
                                psv[:], xv(e, slice(tb * 128, (tb + 1) * 128)),
                                wv(e, slice(VOFF, CL)),
                                start=(e == 0), stop=(e == EB - 1))
                        kb = tt * (TT // 128) + tb
                        nc.scalar.activation(v_hold[:, kb, :], psv[:],
                                             ACT.Copy)
                    if b == 0:
                        wo_feed(3)
                    elif b < b_run - 1 or tt < 2:
                        emit_oproj(b - 1, 1, range(4 * tt, 4 * tt + 4))

                # prefetch next batch's first x tile during attention
                if b + 1 < b_run:
                    xcache[(b + 1, 0)] = load_xtile(b + 1, 0)

                # ---- attention: heads interleaved, PV lags scores ----
                for qt in range(NBT):
                    nkt = (qt + 1) * (TT // 128)
                    ps_o = [ops_o.tile([128, TT], f32, tag=f"o{j}",
                                       name=f"ps_o{j}") for j in range(HPC)]
                    ps_z = [zps.tile([128, TT], f32, tag=f"z{j}",
                                     name=f"ps_z{j}") for j in range(HPC)]
                    pts = {}
                    for kt in range(nkt + 1):
                        if kt < nkt:
                            m = kt - qt * (TT // 128)
                            lo = max(m, 0) * 128
                            for j in range(HPC):
                                ps_s = bps.tile([128, TT], f32, tag="big",
                                                name="ps_s")
                                nc.tensor.matmul(
                                    ps_s[:, lo:],
                                    k_sb[j][:, kt * 128:(kt + 1) * 128],
                                    q_sb[j][:, qt * TT + lo:(qt + 1) * TT],
                                    start=True, stop=True)
                                pt = pt_pool.tile([128, TT], bf16, tag="pt",
                                                  name="pt")
                                nc.scalar.activation(
                                    pt[:, lo:], ps_s[:, lo:], ACT.Exp)
                                if m >= 0:
                                    nc.vector.tensor_mul(
                                        pt[:, lo:lo + 128],
                                        pt[:, lo:lo + 128], tri_sb[:])
                                pts[(j, kt)] = pt
                        if kt > 0:
                            pk = kt - 1
                            lo = max(pk - qt * (TT // 128), 0) * 128
                            for j in range(HPC):
                                pt = pts.pop((j, pk))
                                nc.tensor.matmul(
                                    ps_o[j][:, lo:],
                                    v_hold[:, pk, j * HD:(j + 1) * HD],
                                    pt[:, lo:],
                                    start=(pk == 0), stop=(pk == nkt - 1))
                                nc.tensor.matmul(
                                    ps_z[j][:, lo:],
                                    ones_mat[:],
                                    pt[:, lo:],
                                    start=(pk == 0), stop=(pk == nkt - 1))
                    for j in range(HPC):
                        # Z arrives broadcast across partitions (ones-matrix
                        # stationary), so normalization is recip + mul on DVE
                        zri = eps_pool.tile([128, TT], f32, tag="zri",
                                            name="zri", bufs=2)
                        nc.vector.reciprocal_approx_fast(zri[:], ps_z[j][:])
                        ao = eps_pool.tile([128, TT], bf16, tag="ao",
                                           name="ao", bufs=2)
                        nc.vector.tensor_mul(ao[:], ps_o[j][:], zri[:])
                        # scatter 128-token blocks: global block g of this
                        # batch -> rank g%8, half g//8
                        for s in range(TT // HB):
                            g = qt * (TT // HB) + s
                            h, rr = g // 8, g % 8
                            nc.sync.dma_start(
                                out=agl[b][h][rr * (E // NC_) + j * HD:
                                              rr * (E // NC_) + (j + 1) * HD,
                                              :],
                                in_=ao[:, s * HB:(s + 1) * HB])
                    if qt == 1:
                        nc.gpsimd.collective_compute(
                            "AllToAll", mybir.AluOpType.bypass,
                            replica_groups=[list(range(NC_))],
                            ins=[agl[b][0][:]], outs=[agf[b][0][:]])
                        prefetch_atile(b, 0)

                # fire half 1's A2A, then o_proj half 0 runs under it
                nc.gpsimd.collective_compute(
                    "AllToAll", mybir.AluOpType.bypass,
                    replica_groups=[list(range(NC_))],
                    ins=[agl[b][1][:]], outs=[agf[b][1][:]])
                prefetch_atile(b, 1)
                emit_oproj(b, 0, range(EB))
                if b == b_run - 1 and b > 0:
                    # deferred half of the previous batch's o_proj plugs the
                    # gap while the final half-A2A + a_tile load completes
                    emit_oproj(b - 1, 1, range(8, EB))

            emit_oproj(b_run - 1, 1, range(EB))
    nc.compile()
    return nc


def _prep_inputs(x, freqs, w_qkv, w_o, b_run=B):
    bf16 = ml_dtypes.bfloat16
    xf = np.ascontiguousarray(x, dtype=np.float32).reshape(b_run * T, E)
    xT = np.ascontiguousarray(xf.T.astype(bf16))

    wq = w_qkv[0:E].reshape(H, HD, E)
    wk = w_qkv[E:2 * E].reshape(H, HD, E)
    wvv = w_qkv[2 * E:3 * E].reshape(H, HD, E)
    perm = np.concatenate([np.arange(0, HD, 2), np.arange(1, HD, 2)])
    scale = np.float32(1.0 / np.sqrt(HD))
    wq_p = wq[:, perm, :] * scale
    wk_p = wk[:, perm, :]

    cos = np.cos(freqs.astype(np.float32))
    sin = np.sin(freqs.astype(np.float32))
    cosT = np.ascontiguousarray(cos.T)
    sinT = np.ascontiguousarray(sin.T)
    tri = (np.arange(128)[:, None] <= np.arange(128)[None, :]).astype(bf16)
    w_oT = np.ascontiguousarray(w_o.T.astype(bf16))

    in_maps = []
    for r in range(NC_):
        blocks = []
        for j in range(HPC):
            h = r * HPC + j
            blocks += [wq_p[h].T, wk_p[h].T]
        blocks += [wvv[r * HPC + j].T for j in range(HPC)]
        wqkvT_loc = np.ascontiguousarray(
            np.concatenate(blocks, axis=1).astype(bf16))
        in_maps.append({
            "xT": xT,
            "wqkvT": wqkvT_loc,
            "woT": w_oT,
            "cosT": cosT,
            "sinT": sinT,
            "trimask": tri,
        })
    return in_maps


def kernel(x, freqs, w_qkv, w_o, _trace=False, _b_run=B):
    global _BUILT
    from concourse.bass_utils import run_bass_kernel_spmd

    if _BUILT is None or _BUILT[1] != _b_run:
        _BUILT = (_build(_b_run), _b_run)
    nc = _BUILT[0]

    in_maps = _prep_inputs(np.asarray(x), np.asarray(freqs),
                           np.asarray(w_qkv), np.asarray(w_o), _b_run)
    res = run_bass_kernel_spmd(nc, in_maps, core_ids=list(range(NC_)),
                               trace=_trace)
    # core r owns token block g of each batch iff g%8 == r, stored in its
    # outT at columns [b*BLK + (g//8)*HB, ...)
    out = np.empty((E, _b_run * T), np.float32)
    for r in range(NC_):
        o = res.results[r]["outT"]
        for b in range(_b_run):
            for h in range(2):
                g = 8 * h + r
                out[:, b * T + g * HB:b * T + (g + 1) * HB] = \
                    o[:, b * BLK + h * HB:b * BLK + (h + 1) * HB] \
                    .astype(np.float32)
    out = np.ascontiguousarray(out.T).reshape(_b_run, T, E)
    if _trace:
        kernel.last_results = res
    return out.astype(np.float32, copy=False)
